# revision 74
# baseline (speedup 1.0000x reference)
"""Multi-head attention (B=4, S=2048, H=1024, NH=16) on 8 trn2 NeuronCores.

Sharding: hybrid batch x head-group. Core c owns batch b = c//2 and head
group g = c%2 (heads 8g..8g+7, feature slice g*512:(g+1)*512). Each core
computes its batch's QKV projections for its 512 features, 8 heads of
causal attention, and the partial output projection over its features;
the host sums the 2 partials per batch (plus the bv/bo constant).

Per-core kernel (all matmuls bf16 -> fp32 PSUM):
  qT/kT  [512f, S]  = Wc @ x.T + b   (x shipped pre-transposed bf16,
                                      4 f-tiles of 128, 8 ht accum steps)
  v      [S, 512f]  natural layout, N=512 streams (LDWEIGHTS stays hidden
                    behind the 512-col moving stream; no bias: bv folded
                    into a host-side constant)
  Attention: q-chunks (512) outer, head-pairs hp=0..3 inner, k-tiles (128)
  rotated diagonal-first:
    scoresT[k, q] = kT_h.T @ qT_h  (both heads of the pair via PE row
                                    tiling into one [128, 2, 512] PSUM)
    expT = exp(scoresT/8) per head (no max subtraction: |scores| small)
    ctxT'[65, q] = [v_h | 1].T @ expT  -> rows 0-63 ctx, row 64 denom
    ctxT_all[:, hp, q] = ctxT'[0:64] * bcast(1/denom)
  out_partial[512 rows, 1024] = sum_hp ctxT_all[:, hp].T @ Wo_ft  (4-step
  PSUM accumulation, emitted once all 4 head-pairs of a q-chunk are done,
  deferred to overlap the next q-chunk's attention)
Host: out[b] = out_partial[2b] + out_partial[2b+1] + (bv @ Wo.T + bo).

Scheduling: units are emitted pair-batched (two units' score matmuls
land at the tensor queue head together, then their exps, then the
lagging AV pair, then paced work) so the scalar exp stream gets ~2 units
of runway per round against the in-order tensor queue. Half-size
projection units (4 of the 8 ht accumulation steps) are paced in on a
feed-forward schedule interpolated backwards from each attention pass's
dependency threshold, so guards never force bursts that starve the exp
stream. Softmax normalization runs in two deferred phases (denominator
copy + reciprocal, then broadcast + multiply two units later) to keep
its latency off every queue's critical path.
"""

import sys

for _p in ("/opt/trn_rl_repo", "/root/.axon_site/_ro/trn_rl_repo"):
    if _p not in sys.path:
        sys.path.insert(0, _p)

from collections import deque

import numpy as np
import ml_dtypes

import concourse.bass as bass
import concourse.mybir as mybir
import concourse.tile as tile
from concourse import bacc
from concourse.bass_utils import run_bass_kernel_spmd

BF16 = ml_dtypes.bfloat16
B, S, H, NH, HS = 4, 2048, 1024, 16, 64
NCORES = 8
GF = 512             # features per core (8 heads)
NFT = 4              # f-tiles of 128 per core
RC = 512             # row chunk for projections
NRC = S // RC        # 4 chunks per batch
QC = 512             # q chunk in attention
NQC = S // QC        # 4
NKT = S // 128       # 16 k-tiles per batch
NHP = 4              # head pairs per core
LOOK = 5             # units of lookahead between exp and AV consumption

_COMPILED = {}


def _build_program():
    fp32 = mybir.dt.float32
    bf16 = mybir.dt.bfloat16

    nc = bacc.Bacc("TRN2", target_bir_lowering=False, debug=False,
                   num_devices=NCORES)

    xq = nc.dram_tensor("xq_t", [NRC, 128, 8, RC], bf16,
                        kind="ExternalInput").ap()
    xk = nc.dram_tensor("xk_t", [NRC, 128, 8, RC], bf16,
                        kind="ExternalInput").ap()
    xv = nc.dram_tensor("xv_t", [NRC, 128, 8, RC], bf16,
                        kind="ExternalInput").ap()
    wq = nc.dram_tensor("wq_t", [NFT, 128, 8, 128], bf16,
                        kind="ExternalInput").ap()
    wk = nc.dram_tensor("wk_t", [NFT, 128, 8, 128], bf16,
                        kind="ExternalInput").ap()
    wv = nc.dram_tensor("wv_t", [128, 8, GF], bf16, kind="ExternalInput").ap()
    wo = nc.dram_tensor("wo_t", [128, NFT, H], bf16, kind="ExternalInput").ap()
    bqd = nc.dram_tensor("bq", [128, NFT], fp32, kind="ExternalInput").ap()
    bkd = nc.dram_tensor("bk", [128, NFT], fp32, kind="ExternalInput").ap()
    out_p = nc.dram_tensor("out_p", [S, H], bf16, kind="ExternalOutput").ap()

    with tile.TileContext(nc) as tc:
        with tc.tile_pool(name="singles", bufs=1) as singles:
            wq_sb = singles.tile([128, NFT, 8, 128], bf16, tag="wq")
            wk_sb = singles.tile([128, NFT, 8, 128], bf16, tag="wk")
            wv_sb = singles.tile([128, 8, GF], bf16, tag="wv")
            wo_sb = singles.tile([128, NFT, H], bf16, tag="wo")
            bq_sb = singles.tile([128, NFT], fp32, tag="bq")
            bk_sb = singles.tile([128, NFT], fp32, tag="bk")
            # doubled causal tile: tri2[p, h, f] = 1.0 where p <= f
            tri2 = singles.tile([128, 2, 128], bf16, tag="tri2")
            qT_sb = singles.tile([128, NFT, S], bf16, tag="qT")
            kT_sb = singles.tile([128, NFT, S], bf16, tag="kT")
            # v natural, per 128-row tile and head: [v_h | ones-col]
            v_sb = singles.tile([128, NKT, 8, 65], bf16, tag="v")
            # normalized ctxT per head-pair, persists until out-projection
            ctxT_all = singles.tile([128, NHP, S], bf16, tag="ctxT_all")

            nc.gpsimd.memset(tri2, 1.0)
            for h in range(2):
                nc.gpsimd.affine_select(
                    out=tri2[:, h, :], in_=tri2[:, h, :],
                    pattern=[[1, 128]], compare_op=mybir.AluOpType.is_ge,
                    fill=0.0, base=0, channel_multiplier=-1,
                )
            nc.gpsimd.memset(v_sb[:, :, :, 64], 1.0)

            # PSUM budget (8 banks):
            #   ss: fused score tiles [128, 2, 512] f32 x2   -> 4 banks
            #   cx: ps_ctx h0/h1 [65, 512] f32, single-buf   -> 2 banks
            #   pj: proj/out-proj ring [128, 512] f32 x2     -> 2 banks
            with tc.tile_pool(name="xa", bufs=2) as xpool, \
                 tc.tile_pool(name="ss", bufs=2, space="PSUM") as sspool, \
                 tc.tile_pool(name="cx", bufs=1, space="PSUM") as cxpool, \
                 tc.tile_pool(name="pj", bufs=2, space="PSUM") as pjpool, \
                 tc.tile_pool(name="ex", bufs=12) as epool, \
                 tc.tile_pool(name="nm", bufs=2) as npool, \
                 tc.tile_pool(name="ot", bufs=3) as otpool:

                # ---------- emitters ----------
                x_tiles = {}

                def prefetch_chunk(c):
                    xq_c = xpool.tile([128, 8, RC], bf16, tag="xq",
                                      name="xq_c")
                    xk_c = xpool.tile([128, 8, RC], bf16, tag="xk",
                                      name="xk_c")
                    xv_c = xpool.tile([128, 8, RC], bf16, tag="xv",
                                      name="xv_c")
                    nc.sync.dma_start(out=xq_c, in_=xq[c])
                    nc.sync.dma_start(out=xk_c, in_=xk[c])
                    nc.sync.dma_start(out=xv_c, in_=xv[c])
                    x_tiles[c] = (xq_c, xk_c, xv_c)

                consumed = {}

                def chunk_done(c):
                    # all 12 proj units of chunk c emitted: free slot -> c+2
                    consumed[c] = consumed.get(c, 0) + 1
                    if consumed[c] == 12 and c + 2 < NRC:
                        prefetch_chunk(c + 2)

                half_ps = {}

                def emit_proj(c, kind, idx, half):
                    # half proj unit: 4 of the 8 ht accumulation steps, so a
                    # paced insert stalls the exp stream by <=0.9us
                    xq_c, xk_c, xv_c = x_tiles[c]
                    key = (c, kind, idx)
                    if half == 0:
                        half_ps[key] = pjpool.tile([128, RC], fp32,
                                                   tag="pj", name=f"ps_{kind}")
                    ps = half_ps[key] if half == 0 else half_ps.pop(key)
                    hts = range(4 * half, 4 * half + 4)
                    if kind in ("q", "k"):
                        ft = idx
                        x_c = xq_c if kind == "q" else xk_c
                        w_sb = wq_sb if kind == "q" else wk_sb
                        for ht in hts:
                            nc.tensor.matmul(
                                ps, w_sb[:, ft, ht, :],
                                x_c[:, ht, :],
                                start=(ht == 0), stop=(ht == 7),
                                skip_group_check=True)
                        if half == 1:
                            dst = qT_sb if kind == "q" else kT_sb
                            bias = bq_sb if kind == "q" else bk_sb
                            nc.vector.tensor_scalar_add(
                                dst[:, ft, bass.ts(c, RC)], ps,
                                bias[:, ft:ft + 1])
                    else:  # v
                        rt = idx
                        for ht in hts:
                            nc.tensor.matmul(
                                ps, xv_c[:, ht, bass.ts(rt, 128)],
                                wv_sb[:, ht, :],
                                start=(ht == 0), stop=(ht == 7),
                                skip_group_check=True)
                        if half == 1:
                            nc.vector.tensor_copy(
                                v_sb[:, c * 4 + rt, :, 0:64],
                                ps.rearrange("p (h f) -> p h f", h=8))
                    if half == 1:
                        chunk_done(c)

                def emit_outproj(qc, rt, fo, tail=False):
                    ps_o = pjpool.tile([128, 512], fp32, tag="pj",
                                       name="ps_o")
                    for hp in range(NHP):
                        nc.tensor.matmul(
                            ps_o,
                            ctxT_all[:, hp,
                                     qc * QC + rt * 128:
                                     qc * QC + rt * 128 + 128],
                            wo_sb[:, hp, bass.ts(fo, 512)],
                            start=(hp == 0), stop=(hp == 3))
                    o_sb = otpool.tile([128, 512], bf16, tag="o_sb",
                                       name="o_sb")
                    if tail:
                        # the exp stream is finished by the tail: use the
                        # idle scalar engine so the copies don't queue on
                        # vector behind the final norm chain
                        nc.scalar.activation(
                            o_sb, ps_o,
                            mybir.ActivationFunctionType.Copy)
                    else:
                        nc.vector.tensor_copy(o_sb, ps_o)
                    r0 = qc * QC + rt * 128
                    nc.sync.dma_start(
                        out=out_p[r0:r0 + 128, bass.ts(fo, 512)],
                        in_=o_sb)

                pending = deque()   # deferred out-proj units

                norm_pending = deque()

                def emit_norm(qc, hp, ps_ctx, ready_i):
                    # Phase 1: pull the denominators out of PSUM and start
                    # the reciprocals; the broadcast + mul run deferred
                    # (emit_norm2) so they never block a queue while the
                    # recip is still in flight. The final pass's copies use
                    # the scalar engine, idle once the exp stream is done.
                    last = qc == NQC - 1 and hp == NHP - 1
                    dd, rec = [], []
                    for h in range(2):
                        dd.append(npool.tile([1, QC], fp32, tag=f"dd{h}",
                                             name=f"dd{h}"))
                        if last:
                            nc.scalar.activation(
                                dd[h], ps_ctx[h][64:65, :],
                                mybir.ActivationFunctionType.Copy)
                        else:
                            nc.vector.tensor_copy(dd[h], ps_ctx[h][64:65, :])
                    for h in range(2):
                        rec.append(npool.tile([1, QC], fp32, tag=f"rec{h}",
                                              name=f"rec{h}"))
                        nc.vector.reciprocal_approx_fast(rec[h], dd[h])
                    norm_pending.append((ready_i, qc, hp, ps_ctx, rec))

                def emit_norm2(cur_i, qc, hp, ps_ctx, rec):
                    # broadcast 1/d across 64 partitions via a K=1 matmul
                    # (both heads in concurrent PE column tiles), then the
                    # normalizing mul into ctxT_all
                    bc = []
                    for h in range(2):
                        bc.append(npool.tile([64, QC], fp32, tag=f"bc{h}",
                                             name=f"bc{h}", bufs=1))
                        nc.gpsimd.partition_broadcast(bc[h], rec[h])
                    if qc == NQC - 1 and hp == NHP - 1:
                        # final pass: normalize in rt-quarters so the tail
                        # out-projections start on their own q-slice without
                        # waiting for the full 512-wide multiply
                        for rt in range(4):
                            for h in range(2):
                                nc.vector.tensor_mul(
                                    ctxT_all[64 * h:64 * h + 64, hp,
                                             qc * QC + rt * 128:
                                             qc * QC + rt * 128 + 128],
                                    ps_ctx[h][0:64, bass.ts(rt, 128)],
                                    bc[h][:, bass.ts(rt, 128)])
                    else:
                        for h in range(2):
                            nc.vector.tensor_mul(
                                ctxT_all[64 * h:64 * h + 64, hp,
                                         bass.ts(qc, QC)],
                                ps_ctx[h][0:64, :], bc[h])
                    if hp == NHP - 1:
                        pending.extend((cur_i + 3, qc, rt, fo)
                                       for rt in range(4) for fo in range(2))

                # ---------- global unit stream ----------
                units = []
                for qc in range(NQC):
                    kts = (list(range(4 * qc, 4 * qc + 4)) +
                           list(range(0, 4 * qc)))
                    for hp in range(NHP):
                        if qc == NQC - 1 and hp == NHP - 1:
                            # final pass: diagonal tiles last, so the
                            # closing exp->AV->norm chain handles the
                            # narrowest tiles and the tail shortens
                            kts = (list(range(0, 4 * qc)) +
                                   list(range(4 * qc, 4 * qc + 4)))
                        for j, kt in enumerate(kts):
                            units.append((qc, hp, kt, j == 0,
                                          j == len(kts) - 1, j))

                # req[i]: halves that must be emitted by the time unit i is
                # reached, spread one per unit backwards from each pass
                # boundary's guard threshold so guards never force bursts
                need = {0: 4, 1: 16, 2: 20, 3: 24}
                NU = len(units)
                req = [0] * (NU + 1)
                for ui in range(NU):
                    uqc, uhp, _ukt, ufirst, _ul, _uj = units[ui]
                    if ufirst:
                        req[ui] = 24 * uqc + need[uhp]
                req[NU] = 24 * NRC
                for ui in range(NU - 1, -1, -1):
                    req[ui] = max(req[ui], req[ui + 1] - 1)
                proj_fifo = deque()
                # ramp: order the queue so the first proj units' inputs
                # land first (xq c0 + wq ft0, then k, then v)
                xq_c = xpool.tile([128, 8, RC], bf16, tag="xq", name="xq_c")
                xk_c = xpool.tile([128, 8, RC], bf16, tag="xk", name="xk_c")
                xv_c = xpool.tile([128, 8, RC], bf16, tag="xv", name="xv_c")
                x_tiles[0] = (xq_c, xk_c, xv_c)
                nc.sync.dma_start(out=wq_sb[:, 0], in_=wq[0])
                nc.sync.dma_start(out=xq_c[:, 0:4], in_=xq[0][:, 0:4])
                nc.sync.dma_start(out=xq_c[:, 4:8], in_=xq[0][:, 4:8])
                nc.sync.dma_start(out=wk_sb[:, 0], in_=wk[0])
                nc.sync.dma_start(out=xk_c[:, 0:4], in_=xk[0][:, 0:4])
                nc.sync.dma_start(out=xk_c[:, 4:8], in_=xk[0][:, 4:8])
                for ft in range(1, NFT):
                    nc.sync.dma_start(out=wq_sb[:, ft], in_=wq[ft])
                nc.sync.dma_start(out=wv_sb, in_=wv)
                nc.sync.dma_start(out=xv_c, in_=xv[0])
                nc.sync.dma_start(out=bq_sb, in_=bqd)
                nc.sync.dma_start(out=bk_sb, in_=bkd)
                prefetch_chunk(1)
                for ft in range(1, NFT):
                    nc.sync.dma_start(out=wk_sb[:, ft], in_=wk[ft])
                nc.sync.dma_start(out=wo_sb, in_=wo)
                for c in range(NRC):
                    order = ([("q", 0), ("k", 0)] +
                             [("v", rt) for rt in range(4)] +
                             [(kd, ft) for ft in range(1, NFT)
                              for kd in ("q", "k")])
                    for kd, idx in order:
                        proj_fifo.append((c, kd, idx, 0))
                        proj_fifo.append((c, kd, idx, 1))
                emitted = 0

                exps = {}
                ps_ctx = None
                SLEAD = 1   # scores/exp emitted this many units ahead of
                            # the AV/insert cursor so paced proj units never
                            # stall the scalar exp stream
                def emit_scores(si):
                    nonlocal emitted
                    qc, hp, kt, first, last, _j = units[si]
                    # correctness guard: everything this (qc, hp) pass
                    # depends on must be emitted before its first unit
                    if first:
                        while proj_fifo and emitted < 24 * qc + need[hp]:
                            emit_proj(*proj_fifo.popleft())
                            emitted += 1
                    jt = kt - 4 * qc       # >=0 on diagonal tiles
                    vs = max(jt, 0) * 128  # valid q start in chunk
                    q0 = qc * QC
                    k0 = kt * 128
                    ps_s = sspool.tile([128, 2, QC], fp32, tag="ss",
                                       name="ps_s")
                    for h in range(2):
                        hh = 2 * hp + h
                        ft, fo = divmod(hh, 2)
                        hp_sl = slice(64 * fo, 64 * fo + 64)
                        nc.tensor.matmul(
                            ps_s[:, h, vs:],
                            kT_sb[hp_sl, ft, k0:k0 + 128],
                            qT_sb[hp_sl, ft, q0 + vs:q0 + QC],
                            start=True, stop=True,
                            tile_position=(64 * fo, 0))
                    # fused exp over both heads (one ACT per unit)
                    e_t = epool.tile([128, 2, QC], bf16, tag="e",
                                     name="e_t")
                    nc.scalar.activation(
                        e_t[:, :, vs:], ps_s[:, :, vs:],
                        mybir.ActivationFunctionType.Exp,
                        scale=0.125)
                    if jt >= 0:
                        nc.vector.tensor_mul(
                            e_t[:, :, vs:vs + 128],
                            e_t[:, :, vs:vs + 128], tri2)
                    exps[(qc, hp, kt)] = e_t

                def emit_av(i):
                    nonlocal ps_ctx
                    pqc, php, pkt, pfirst, plast, _pj = units[i - LOOK]
                    pvs = max(pkt - 4 * pqc, 0) * 128
                    if pfirst:
                        ps_ctx = [cxpool.tile([65, QC], fp32,
                                              tag=f"ctx{h}",
                                              name=f"ps_ctx{h}")
                                  for h in range(2)]
                    e_prev = exps.pop((pqc, php, pkt))
                    for h in range(2):
                        nc.tensor.matmul(
                            ps_ctx[h][:, pvs:],
                            v_sb[:, pkt, 2 * php + h, :],
                            e_prev[:, h, pvs:],
                            start=pfirst, stop=plast,
                            skip_group_check=True)
                    if plast:
                        emit_norm(pqc, php, ps_ctx, i + 4)

                # pair-batched emission: two units' scores land together so
                # the exp stream gets ~2 units of runway per round
                for i in range(-SLEAD, len(units) + LOOK, 2):
                    for d in (0, 1):
                        si = i + d + SLEAD
                        if 0 <= si < len(units):
                            emit_scores(si)
                    for d in (0, 1):
                        ii = i + d
                        if LOOK <= ii < len(units) + LOOK and ii >= 0:
                            emit_av(ii)
                    for d in (0, 1):
                        ii = i + d
                        if ii < 0:
                            continue
                        if norm_pending and norm_pending[0][0] + 2 <= ii:
                            emit_norm2(ii, *norm_pending.popleft()[1:])
                        if ii >= len(units):
                            if pending and pending[0][0] <= ii:
                                emit_outproj(*pending.popleft()[1:],
                                             tail=True)
                            continue
                        if proj_fifo and (emitted < req[min(ii + 2, NU)] + 1
                                          or units[ii][0] == 0
                                          or ii % 2 == 0):
                            emit_proj(*proj_fifo.popleft())
                            emitted += 1
                        elif pending and pending[0][0] <= ii:
                            emit_outproj(*pending.popleft()[1:])
                            if not proj_fifo and len(pending) > 8 \
                                    and pending[0][0] <= ii:
                                emit_outproj(*pending.popleft()[1:])

                while norm_pending:
                    emit_norm2(10 ** 9, *norm_pending.popleft()[1:])
                while pending:
                    emit_outproj(*pending.popleft()[1:], tail=True)

    nc.compile()
    return nc


def _prep_inputs(query, key, value, Wq, bq, Wk, bk, Wv, bv, Wo, bo):
    f32 = np.float32

    def blocked(x):
        # [S, H] -> [c, p, ht, r] with S = c*512 + r, H = ht*128 + p
        return np.ascontiguousarray(
            x.reshape(NRC, RC, 8, 128).transpose(0, 3, 2, 1)).astype(BF16)

    xb = [(blocked(query[b]), blocked(key[b]), blocked(value[b]))
          for b in range(B)]
    in_maps = []
    for c in range(NCORES):
        b, g = divmod(c, 2)
        fs = slice(g * GF, (g + 1) * GF)
        in_maps.append({
            "xq_t": xb[b][0], "xk_t": xb[b][1], "xv_t": xb[b][2],
            "wq_t": np.ascontiguousarray(
                Wq[fs].T.reshape(8, 128, NFT, 128).transpose(2, 1, 0, 3)
            ).astype(BF16),
            "wk_t": np.ascontiguousarray(
                Wk[fs].T.reshape(8, 128, NFT, 128).transpose(2, 1, 0, 3)
            ).astype(BF16),
            "wv_t": np.ascontiguousarray(
                Wv[fs].T.reshape(8, 128, GF).transpose(1, 0, 2)
            ).astype(BF16),
            "wo_t": np.ascontiguousarray(
                Wo[:, fs].T.reshape(NFT, 128, H).transpose(1, 0, 2)
            ).astype(BF16),
            "bq": np.ascontiguousarray(
                bq[fs].reshape(NFT, 128).T).astype(f32),
            "bk": np.ascontiguousarray(
                bk[fs].reshape(NFT, 128).T).astype(f32),
        })
    const = (bv.astype(f32) @ Wo.T.astype(f32) + bo.astype(f32))
    return in_maps, const


def kernel(query, key, value, causal_mask, Wq, bq, Wk, bk, Wv, bv, Wo, bo,
           _trace=False, _return_res=False):
    if "nc" not in _COMPILED:
        _COMPILED["nc"] = _build_program()
    nc = _COMPILED["nc"]
    in_maps, const = _prep_inputs(query, key, value, Wq, bq, Wk, bk,
                                  Wv, bv, Wo, bo)
    res = run_bass_kernel_spmd(nc, in_maps, list(range(NCORES)), trace=_trace)
    out = np.empty((B, S, H), np.float32)
    for b in range(B):
        out[b] = res.results[2 * b]["out_p"].astype(np.float32)
        out[b] += res.results[2 * b + 1]["out_p"]
        out[b] += const
    if _return_res:
        return out, res
    return out


# revision 75
# speedup vs baseline: 1.0433x; 1.0433x over previous
"""Multi-head attention (B=4, S=2048, H=1024, NH=16) on 8 trn2 NeuronCores.

Sharding: hybrid batch x head-group. Core c owns batch b = c//2 and head
group g = c%2 (heads 8g..8g+7, feature slice g*512:(g+1)*512). Each core
computes its batch's QKV projections for its 512 features, 8 heads of
causal attention, and the partial output projection over its features;
the host sums the 2 partials per batch (plus the bv/bo constant).

Per-core kernel (all matmuls bf16 -> fp32 PSUM):
  qT/kT  [512f, S]  = Wc @ x.T + b   (x shipped pre-transposed bf16,
                                      4 f-tiles of 128, 8 ht accum steps)
  v      [S, 512f]  natural layout, N=512 streams (LDWEIGHTS stays hidden
                    behind the 512-col moving stream; no bias: bv folded
                    into a host-side constant)
  Attention: q-chunks (512) outer, head-pairs hp=0..3 inner, k-tiles (128)
  rotated diagonal-first:
    scoresT[k, q] = kT_h.T @ qT_h  (both heads of the pair via PE row
                                    tiling into one [128, 2, 512] PSUM)
    expT = exp(scoresT/8) per head (no max subtraction: |scores| small)
    ctxT'[65, q] = [v_h | 1].T @ expT  -> rows 0-63 ctx, row 64 denom
    ctxT_all[:, hp, q] = ctxT'[0:64] * bcast(1/denom)
  out_partial[512 rows, 1024] = sum_hp ctxT_all[:, hp].T @ Wo_ft  (4-step
  PSUM accumulation, emitted once all 4 head-pairs of a q-chunk are done,
  deferred to overlap the next q-chunk's attention)
Host: out[b] = out_partial[2b] + out_partial[2b+1] + (bv @ Wo.T + bo).

Scheduling: units are emitted pair-batched (two units' score matmuls
land at the tensor queue head together, then their exps, then the
lagging AV pair, then paced work) so the scalar exp stream gets ~2 units
of runway per round against the in-order tensor queue. Half-size
projection units (4 of the 8 ht accumulation steps) are paced in on a
feed-forward schedule interpolated backwards from each attention pass's
dependency threshold, so guards never force bursts that starve the exp
stream. Softmax normalization runs in two deferred phases (denominator
copy + reciprocal, then broadcast + multiply two units later) to keep
its latency off every queue's critical path.
"""

import sys

for _p in ("/opt/trn_rl_repo", "/root/.axon_site/_ro/trn_rl_repo"):
    if _p not in sys.path:
        sys.path.insert(0, _p)

from collections import deque

import numpy as np
import ml_dtypes

import concourse.bass as bass
import concourse.mybir as mybir
import concourse.tile as tile
from concourse import bacc
from concourse.bass_utils import run_bass_kernel_spmd

BF16 = ml_dtypes.bfloat16
B, S, H, NH, HS = 4, 2048, 1024, 16, 64
NCORES = 8
GF = 512             # features per core (8 heads)
NFT = 4              # f-tiles of 128 per core
RC = 512             # row chunk for projections
NRC = S // RC        # 4 chunks per batch
QC = 512             # q chunk in attention
NQC = S // QC        # 4
NKT = S // 128       # 16 k-tiles per batch
NHP = 4              # head pairs per core
LOOK = 6             # units of lookahead between exp and AV consumption

_COMPILED = {}


def _build_program():
    fp32 = mybir.dt.float32
    bf16 = mybir.dt.bfloat16

    nc = bacc.Bacc("TRN2", target_bir_lowering=False, debug=False,
                   num_devices=NCORES)

    xq = nc.dram_tensor("xq_t", [NRC, 128, 8, RC], bf16,
                        kind="ExternalInput").ap()
    xk = nc.dram_tensor("xk_t", [NRC, 128, 8, RC], bf16,
                        kind="ExternalInput").ap()
    xv = nc.dram_tensor("xv_t", [NRC, 128, 8, RC], bf16,
                        kind="ExternalInput").ap()
    wq = nc.dram_tensor("wq_t", [NFT, 128, 8, 128], bf16,
                        kind="ExternalInput").ap()
    wk = nc.dram_tensor("wk_t", [NFT, 128, 8, 128], bf16,
                        kind="ExternalInput").ap()
    wv = nc.dram_tensor("wv_t", [128, 8, GF], bf16, kind="ExternalInput").ap()
    wo = nc.dram_tensor("wo_t", [128, NFT, H], bf16, kind="ExternalInput").ap()
    bqd = nc.dram_tensor("bq", [128, NFT], fp32, kind="ExternalInput").ap()
    bkd = nc.dram_tensor("bk", [128, NFT], fp32, kind="ExternalInput").ap()
    out_p = nc.dram_tensor("out_p", [S, H], bf16, kind="ExternalOutput").ap()

    with tile.TileContext(nc) as tc:
        with tc.tile_pool(name="singles", bufs=1) as singles:
            wq_sb = singles.tile([128, NFT, 8, 128], bf16, tag="wq")
            wk_sb = singles.tile([128, NFT, 8, 128], bf16, tag="wk")
            wv_sb = singles.tile([128, 8, GF], bf16, tag="wv")
            wo_sb = singles.tile([128, NFT, H], bf16, tag="wo")
            bq_sb = singles.tile([128, NFT], fp32, tag="bq")
            bk_sb = singles.tile([128, NFT], fp32, tag="bk")
            # doubled causal tile: tri2[p, h, f] = 1.0 where p <= f
            tri2 = singles.tile([128, 2, 128], bf16, tag="tri2")
            qT_sb = singles.tile([128, NFT, S], bf16, tag="qT")
            kT_sb = singles.tile([128, NFT, S], bf16, tag="kT")
            # v natural, per 128-row tile and head: [v_h | ones-col]
            v_sb = singles.tile([128, NKT, 8, 65], bf16, tag="v")
            # normalized ctxT per head-pair, persists until out-projection
            ctxT_all = singles.tile([128, NHP, S], bf16, tag="ctxT_all")

            nc.gpsimd.memset(tri2, 1.0)
            for h in range(2):
                nc.gpsimd.affine_select(
                    out=tri2[:, h, :], in_=tri2[:, h, :],
                    pattern=[[1, 128]], compare_op=mybir.AluOpType.is_ge,
                    fill=0.0, base=0, channel_multiplier=-1,
                )
            nc.gpsimd.memset(v_sb[:, :, :, 64], 1.0)

            # PSUM budget (8 banks):
            #   ss: fused score tiles [128, 2, 512] f32 x2   -> 4 banks
            #   cx: ps_ctx h0/h1 [65, 512] f32, single-buf   -> 2 banks
            #   pj: proj/out-proj ring [128, 512] f32 x2     -> 2 banks
            with tc.tile_pool(name="xa", bufs=2) as xpool, \
                 tc.tile_pool(name="ss", bufs=2, space="PSUM") as sspool, \
                 tc.tile_pool(name="cx", bufs=1, space="PSUM") as cxpool, \
                 tc.tile_pool(name="pj", bufs=2, space="PSUM") as pjpool, \
                 tc.tile_pool(name="ex", bufs=12) as epool, \
                 tc.tile_pool(name="nm", bufs=2) as npool, \
                 tc.tile_pool(name="ot", bufs=3) as otpool:

                # ---------- emitters ----------
                x_tiles = {}

                def prefetch_chunk(c):
                    xq_c = xpool.tile([128, 8, RC], bf16, tag="xq",
                                      name="xq_c")
                    xk_c = xpool.tile([128, 8, RC], bf16, tag="xk",
                                      name="xk_c")
                    xv_c = xpool.tile([128, 8, RC], bf16, tag="xv",
                                      name="xv_c")
                    nc.sync.dma_start(out=xq_c, in_=xq[c])
                    nc.sync.dma_start(out=xk_c, in_=xk[c])
                    nc.sync.dma_start(out=xv_c, in_=xv[c])
                    x_tiles[c] = (xq_c, xk_c, xv_c)

                consumed = {}

                def chunk_done(c):
                    # all 12 proj units of chunk c emitted: free slot -> c+2
                    consumed[c] = consumed.get(c, 0) + 1
                    if consumed[c] == 12 and c + 2 < NRC:
                        prefetch_chunk(c + 2)

                half_ps = {}

                def emit_proj(c, kind, idx, half):
                    # half proj unit: 4 of the 8 ht accumulation steps, so a
                    # paced insert stalls the exp stream by <=0.9us
                    xq_c, xk_c, xv_c = x_tiles[c]
                    key = (c, kind, idx)
                    if half == 0:
                        half_ps[key] = pjpool.tile([128, RC], fp32,
                                                   tag="pj", name=f"ps_{kind}")
                    ps = half_ps[key] if half == 0 else half_ps.pop(key)
                    hts = range(4 * half, 4 * half + 4)
                    if kind in ("q", "k"):
                        ft = idx
                        x_c = xq_c if kind == "q" else xk_c
                        w_sb = wq_sb if kind == "q" else wk_sb
                        for ht in hts:
                            nc.tensor.matmul(
                                ps, w_sb[:, ft, ht, :],
                                x_c[:, ht, :],
                                start=(ht == 0), stop=(ht == 7),
                                skip_group_check=True)
                        if half == 1:
                            dst = qT_sb if kind == "q" else kT_sb
                            bias = bq_sb if kind == "q" else bk_sb
                            nc.vector.tensor_scalar_add(
                                dst[:, ft, bass.ts(c, RC)], ps,
                                bias[:, ft:ft + 1])
                    else:  # v
                        rt = idx
                        for ht in hts:
                            nc.tensor.matmul(
                                ps, xv_c[:, ht, bass.ts(rt, 128)],
                                wv_sb[:, ht, :],
                                start=(ht == 0), stop=(ht == 7),
                                skip_group_check=True)
                        if half == 1:
                            nc.vector.tensor_copy(
                                v_sb[:, c * 4 + rt, :, 0:64],
                                ps.rearrange("p (h f) -> p h f", h=8))
                    if half == 1:
                        chunk_done(c)

                def emit_outproj(qc, rt, fo, tail=False):
                    ps_o = pjpool.tile([128, 512], fp32, tag="pj",
                                       name="ps_o")
                    for hp in range(NHP):
                        nc.tensor.matmul(
                            ps_o,
                            ctxT_all[:, hp,
                                     qc * QC + rt * 128:
                                     qc * QC + rt * 128 + 128],
                            wo_sb[:, hp, bass.ts(fo, 512)],
                            start=(hp == 0), stop=(hp == 3))
                    o_sb = otpool.tile([128, 512], bf16, tag="o_sb",
                                       name="o_sb")
                    if tail:
                        # the exp stream is finished by the tail: use the
                        # idle scalar engine so the copies don't queue on
                        # vector behind the final norm chain
                        nc.scalar.activation(
                            o_sb, ps_o,
                            mybir.ActivationFunctionType.Copy)
                    else:
                        nc.vector.tensor_copy(o_sb, ps_o)
                    r0 = qc * QC + rt * 128
                    nc.sync.dma_start(
                        out=out_p[r0:r0 + 128, bass.ts(fo, 512)],
                        in_=o_sb)

                pending = deque()   # deferred out-proj units

                norm_pending = deque()

                def emit_norm(qc, hp, ps_ctx, ready_i):
                    # Phase 1: pull the denominators out of PSUM and start
                    # the reciprocals; the broadcast + mul run deferred
                    # (emit_norm2) so they never block a queue while the
                    # recip is still in flight. The final pass's copies use
                    # the scalar engine, idle once the exp stream is done.
                    last = qc == NQC - 1 and hp == NHP - 1
                    dd, rec = [], []
                    for h in range(2):
                        dd.append(npool.tile([1, QC], fp32, tag=f"dd{h}",
                                             name=f"dd{h}"))
                        if last:
                            nc.scalar.activation(
                                dd[h], ps_ctx[h][64:65, :],
                                mybir.ActivationFunctionType.Copy)
                        else:
                            nc.vector.tensor_copy(dd[h], ps_ctx[h][64:65, :])
                    for h in range(2):
                        rec.append(npool.tile([1, QC], fp32, tag=f"rec{h}",
                                              name=f"rec{h}"))
                        nc.vector.reciprocal_approx_fast(rec[h], dd[h])
                    norm_pending.append((ready_i, qc, hp, ps_ctx, rec))

                def emit_norm2(cur_i, qc, hp, ps_ctx, rec):
                    # broadcast 1/d across 64 partitions via a K=1 matmul
                    # (both heads in concurrent PE column tiles), then the
                    # normalizing mul into ctxT_all
                    bc = []
                    for h in range(2):
                        bc.append(npool.tile([64, QC], fp32, tag=f"bc{h}",
                                             name=f"bc{h}", bufs=1))
                        nc.gpsimd.partition_broadcast(bc[h], rec[h])
                    if qc == NQC - 1 and hp == NHP - 1:
                        # final pass: normalize in rt-quarters so the tail
                        # out-projections start on their own q-slice without
                        # waiting for the full 512-wide multiply
                        for rt in range(4):
                            for h in range(2):
                                nc.vector.tensor_mul(
                                    ctxT_all[64 * h:64 * h + 64, hp,
                                             qc * QC + rt * 128:
                                             qc * QC + rt * 128 + 128],
                                    ps_ctx[h][0:64, bass.ts(rt, 128)],
                                    bc[h][:, bass.ts(rt, 128)])
                    else:
                        for h in range(2):
                            nc.vector.tensor_mul(
                                ctxT_all[64 * h:64 * h + 64, hp,
                                         bass.ts(qc, QC)],
                                ps_ctx[h][0:64, :], bc[h])
                    if hp == NHP - 1:
                        pending.extend((cur_i + 3, qc, rt, fo)
                                       for rt in range(4) for fo in range(2))

                # ---------- global unit stream ----------
                units = []
                for qc in range(NQC):
                    kts = (list(range(4 * qc, 4 * qc + 4)) +
                           list(range(0, 4 * qc)))
                    for hp in range(NHP):
                        if qc == NQC - 1 and hp == NHP - 1:
                            # final pass: diagonal tiles last, so the
                            # closing exp->AV->norm chain handles the
                            # narrowest tiles and the tail shortens
                            kts = (list(range(0, 4 * qc)) +
                                   list(range(4 * qc, 4 * qc + 4)))
                        for j, kt in enumerate(kts):
                            units.append((qc, hp, kt, j == 0,
                                          j == len(kts) - 1, j))

                # req[i]: halves that must be emitted by the time unit i is
                # reached, spread one per unit backwards from each pass
                # boundary's guard threshold so guards never force bursts
                need = {0: 4, 1: 16, 2: 20, 3: 24}
                NU = len(units)
                req = [0] * (NU + 1)
                for ui in range(NU):
                    uqc, uhp, _ukt, ufirst, _ul, _uj = units[ui]
                    if ufirst:
                        req[ui] = 24 * uqc + need[uhp]
                req[NU] = 24 * NRC
                for ui in range(NU - 1, -1, -1):
                    req[ui] = max(req[ui], req[ui + 1] - 1)
                proj_fifo = deque()
                # ramp: order the queue so the first proj units' inputs
                # land first (xq c0 + wq ft0, then k, then v)
                xq_c = xpool.tile([128, 8, RC], bf16, tag="xq", name="xq_c")
                xk_c = xpool.tile([128, 8, RC], bf16, tag="xk", name="xk_c")
                xv_c = xpool.tile([128, 8, RC], bf16, tag="xv", name="xv_c")
                x_tiles[0] = (xq_c, xk_c, xv_c)
                nc.sync.dma_start(out=wq_sb[:, 0], in_=wq[0])
                nc.sync.dma_start(out=xq_c[:, 0:4], in_=xq[0][:, 0:4])
                nc.sync.dma_start(out=xq_c[:, 4:8], in_=xq[0][:, 4:8])
                nc.sync.dma_start(out=wk_sb[:, 0], in_=wk[0])
                nc.sync.dma_start(out=xk_c[:, 0:4], in_=xk[0][:, 0:4])
                nc.sync.dma_start(out=xk_c[:, 4:8], in_=xk[0][:, 4:8])
                for ft in range(1, NFT):
                    nc.sync.dma_start(out=wq_sb[:, ft], in_=wq[ft])
                nc.sync.dma_start(out=wv_sb, in_=wv)
                nc.sync.dma_start(out=xv_c, in_=xv[0])
                nc.sync.dma_start(out=bq_sb, in_=bqd)
                nc.sync.dma_start(out=bk_sb, in_=bkd)
                prefetch_chunk(1)
                for ft in range(1, NFT):
                    nc.sync.dma_start(out=wk_sb[:, ft], in_=wk[ft])
                nc.sync.dma_start(out=wo_sb, in_=wo)
                for c in range(NRC):
                    order = ([("q", 0), ("k", 0)] +
                             [("v", rt) for rt in range(4)] +
                             [(kd, ft) for ft in range(1, NFT)
                              for kd in ("q", "k")])
                    for kd, idx in order:
                        proj_fifo.append((c, kd, idx, 0))
                        proj_fifo.append((c, kd, idx, 1))
                emitted = 0

                exps = {}
                ps_ctx = None
                SLEAD = 1   # scores/exp emitted this many units ahead of
                            # the AV/insert cursor so paced proj units never
                            # stall the scalar exp stream
                def emit_scores(si):
                    nonlocal emitted
                    qc, hp, kt, first, last, _j = units[si]
                    # correctness guard: everything this (qc, hp) pass
                    # depends on must be emitted before its first unit
                    if first:
                        while proj_fifo and emitted < 24 * qc + need[hp]:
                            emit_proj(*proj_fifo.popleft())
                            emitted += 1
                    jt = kt - 4 * qc       # >=0 on diagonal tiles
                    vs = max(jt, 0) * 128  # valid q start in chunk
                    q0 = qc * QC
                    k0 = kt * 128
                    ps_s = sspool.tile([128, 2, QC], fp32, tag="ss",
                                       name="ps_s")
                    for h in range(2):
                        hh = 2 * hp + h
                        ft, fo = divmod(hh, 2)
                        hp_sl = slice(64 * fo, 64 * fo + 64)
                        nc.tensor.matmul(
                            ps_s[:, h, vs:],
                            kT_sb[hp_sl, ft, k0:k0 + 128],
                            qT_sb[hp_sl, ft, q0 + vs:q0 + QC],
                            start=True, stop=True,
                            tile_position=(64 * fo, 0))
                    # fused exp over both heads (one ACT per unit)
                    e_t = epool.tile([128, 2, QC], bf16, tag="e",
                                     name="e_t")
                    nc.scalar.activation(
                        e_t[:, :, vs:], ps_s[:, :, vs:],
                        mybir.ActivationFunctionType.Exp,
                        scale=0.125)
                    if jt >= 0:
                        nc.vector.tensor_mul(
                            e_t[:, :, vs:vs + 128],
                            e_t[:, :, vs:vs + 128], tri2)
                    exps[(qc, hp, kt)] = e_t

                def emit_av(i):
                    nonlocal ps_ctx
                    pqc, php, pkt, pfirst, plast, _pj = units[i - LOOK]
                    pvs = max(pkt - 4 * pqc, 0) * 128
                    if pfirst:
                        ps_ctx = [cxpool.tile([65, QC], fp32,
                                              tag=f"ctx{h}",
                                              name=f"ps_ctx{h}")
                                  for h in range(2)]
                    e_prev = exps.pop((pqc, php, pkt))
                    for h in range(2):
                        nc.tensor.matmul(
                            ps_ctx[h][:, pvs:],
                            v_sb[:, pkt, 2 * php + h, :],
                            e_prev[:, h, pvs:],
                            start=pfirst, stop=plast,
                            skip_group_check=True)
                    if plast:
                        emit_norm(pqc, php, ps_ctx, i + 4)

                # pair-batched emission: two units' scores land together so
                # the exp stream gets ~2 units of runway per round
                for i in range(-SLEAD, len(units) + LOOK, 2):
                    for d in (0, 1):
                        si = i + d + SLEAD
                        if 0 <= si < len(units):
                            emit_scores(si)
                    for d in (0, 1):
                        ii = i + d
                        if LOOK <= ii < len(units) + LOOK and ii >= 0:
                            emit_av(ii)
                    for d in (0, 1):
                        ii = i + d
                        if ii < 0:
                            continue
                        if norm_pending and norm_pending[0][0] + 2 <= ii:
                            emit_norm2(ii, *norm_pending.popleft()[1:])
                        if ii >= len(units):
                            if pending and pending[0][0] <= ii:
                                emit_outproj(*pending.popleft()[1:],
                                             tail=True)
                            continue
                        if proj_fifo and (emitted < req[min(ii + 2, NU)] + 1
                                          or units[ii][0] == 0
                                          or ii % 2 == 0):
                            emit_proj(*proj_fifo.popleft())
                            emitted += 1
                        elif pending and pending[0][0] <= ii:
                            emit_outproj(*pending.popleft()[1:])
                            if not proj_fifo and len(pending) > 8 \
                                    and pending[0][0] <= ii:
                                emit_outproj(*pending.popleft()[1:])

                while norm_pending:
                    emit_norm2(10 ** 9, *norm_pending.popleft()[1:])
                while pending:
                    emit_outproj(*pending.popleft()[1:], tail=True)

    nc.compile()
    return nc


def _prep_inputs(query, key, value, Wq, bq, Wk, bk, Wv, bv, Wo, bo):
    f32 = np.float32

    def blocked(x):
        # [S, H] -> [c, p, ht, r] with S = c*512 + r, H = ht*128 + p
        return np.ascontiguousarray(
            x.reshape(NRC, RC, 8, 128).transpose(0, 3, 2, 1)).astype(BF16)

    xb = [(blocked(query[b]), blocked(key[b]), blocked(value[b]))
          for b in range(B)]
    in_maps = []
    for c in range(NCORES):
        b, g = divmod(c, 2)
        fs = slice(g * GF, (g + 1) * GF)
        in_maps.append({
            "xq_t": xb[b][0], "xk_t": xb[b][1], "xv_t": xb[b][2],
            "wq_t": np.ascontiguousarray(
                Wq[fs].T.reshape(8, 128, NFT, 128).transpose(2, 1, 0, 3)
            ).astype(BF16),
            "wk_t": np.ascontiguousarray(
                Wk[fs].T.reshape(8, 128, NFT, 128).transpose(2, 1, 0, 3)
            ).astype(BF16),
            "wv_t": np.ascontiguousarray(
                Wv[fs].T.reshape(8, 128, GF).transpose(1, 0, 2)
            ).astype(BF16),
            "wo_t": np.ascontiguousarray(
                Wo[:, fs].T.reshape(NFT, 128, H).transpose(1, 0, 2)
            ).astype(BF16),
            "bq": np.ascontiguousarray(
                bq[fs].reshape(NFT, 128).T).astype(f32),
            "bk": np.ascontiguousarray(
                bk[fs].reshape(NFT, 128).T).astype(f32),
        })
    const = (bv.astype(f32) @ Wo.T.astype(f32) + bo.astype(f32))
    return in_maps, const


def kernel(query, key, value, causal_mask, Wq, bq, Wk, bk, Wv, bv, Wo, bo,
           _trace=False, _return_res=False):
    if "nc" not in _COMPILED:
        _COMPILED["nc"] = _build_program()
    nc = _COMPILED["nc"]
    in_maps, const = _prep_inputs(query, key, value, Wq, bq, Wk, bk,
                                  Wv, bv, Wo, bo)
    res = run_bass_kernel_spmd(nc, in_maps, list(range(NCORES)), trace=_trace)
    out = np.empty((B, S, H), np.float32)
    for b in range(B):
        out[b] = res.results[2 * b]["out_p"].astype(np.float32)
        out[b] += res.results[2 * b + 1]["out_p"]
        out[b] += const
    if _return_res:
        return out, res
    return out
